# revision 29
# baseline (speedup 1.0000x reference)
"""Trainium2 Bass kernel for nn_MultiHeadAttention_57037165691224.

Full-input contract: kernel(**inputs) takes the unsharded inputs from
setup_inputs() and returns the full [2, 2048, 1024] float32 output.

Sharding (8 cores): core c handles batch b = c//4 and head group g = c%4
(4 heads of 64 dims each). Wq/Wk/Wv are split column-wise by head group,
Wo row-wise; each core emits a partial output projection for its batch,
summed on the host (tensor-parallel heads x data-parallel batch).

Per-core dataflow (all layouts chosen so no on-device transposes needed):
  qT/kT/vT [D=1024, S=2048] bf16 (host-transposed)
  QgT,KgT [Dh=256, S]  = (Wg^T x q) computed as lhsT=W chunk, rhs=qT chunk
  Vg      [S, 4*65]    = (v x Wvg) with a ones column per head (65th col)
  scores^T tile [k=128, q=512] = KgT_h^T-slice x QgT_h (K=64 contraction)
  P = exp(scores*1/8) via ScalarE (no max subtraction; scores are small),
      multiplicative 0/1 causal mask on diagonal tiles only
  O~^T [65, 512] PSUM = Vg_h^T x P^T accumulated over k tiles; row 64 is
      the softmax denominator (ones-column trick)
  normalize: reciprocal + partition_broadcast + DVE mul -> concat^T bf16
  out: O^T partial [1024, 2048] f32 = Wo_g^T x concat^T
"""

import numpy as np
import ml_dtypes

import concourse.bass as bass
import concourse.mybir as mybir
import concourse.tile as tile
from concourse import bacc
from concourse.bass import ts
from concourse.bass_utils import run_bass_kernel_spmd

BF16 = mybir.dt.bfloat16
F32 = mybir.dt.float32
bf16 = ml_dtypes.bfloat16

B, S, D, H = 2, 2048, 1024, 16
DK = 64          # head dim
G = 4            # heads per core
DH = G * DK      # 256 cols per head group
DHA = G * (DK + 1)  # V augmented with ones column per head
N_CORES = 8
SC = 512         # q-chunk (moving free dim / PSUM bank)
KT = S // 128    # 16 k-tiles
NJ = S // SC     # 4 q-chunks
ND = D // 128    # 8 contraction tiles over D
SCALE = 1.0 / 8.0


def build_program(causal: bool, reps: int = 1, with_bias: bool = True):
    """Build + compile the SPMD program. Same program runs on all 8 cores;
    per-core behavior differs only via input data."""
    nc = bacc.Bacc("TRN2", target_bir_lowering=False, debug=False,
                   num_devices=N_CORES)

    qT = nc.dram_tensor("qT", [D, S], BF16, kind="ExternalInput").ap()
    kT = nc.dram_tensor("kT", [D, S], BF16, kind="ExternalInput").ap()
    vT = nc.dram_tensor("vT", [D, S], BF16, kind="ExternalInput").ap()
    wq = nc.dram_tensor("wq", [D, DH], BF16, kind="ExternalInput").ap()
    wk = nc.dram_tensor("wk", [D, DH], BF16, kind="ExternalInput").ap()
    wv = nc.dram_tensor("wv", [D, DH], BF16, kind="ExternalInput").ap()
    wo = nc.dram_tensor("wo", [DH, D], BF16, kind="ExternalInput").ap()
    bqr = nc.dram_tensor("bqr", [1, DH], BF16, kind="ExternalInput").ap()
    bkr = nc.dram_tensor("bkr", [1, DH], BF16, kind="ExternalInput").ap()
    bvr = nc.dram_tensor("bvr", [1, DH], BF16, kind="ExternalInput").ap()
    msk = nc.dram_tensor("msk", [4, 128, SC], BF16, kind="ExternalInput").ap()
    oT = nc.dram_tensor("oT", [D, S], BF16, kind="ExternalOutput").ap()

    with tile.TileContext(nc) as tc:
        _trace_body(tc, nc, causal, reps, with_bias, qT, kT, vT, wq, wk, wv, wo, bqr, bkr, bvr, msk, oT)
    nc.compile()
    return nc


def _mark(nc, phase):
    # phase attribution for TimelineSim analysis: record the next I-number
    marks = getattr(nc, "_phase_marks", None)
    if marks is None:
        marks = nc._phase_marks = []
    nid = int(nc.get_next_instruction_name().split("-")[1])
    marks.append((nid, phase))


def _trace_body(tc, nc, causal, reps, with_bias,
                qT, kT, vT, wq, wk, wv, wo, bqr, bkr, bvr, msk, oT):
    from contextlib import ExitStack
    with ExitStack() as ctx:
        ep = ctx.enter_context

        # ---- persistent pools -------------------------------------------
        p_w = ep(tc.tile_pool(name="w", bufs=3))        # wq/wk/wv tiles
        p_wo = ep(tc.tile_pool(name="wo", bufs=1))
        p_bias = ep(tc.tile_pool(name="bias", bufs=3))
        p_msk = ep(tc.tile_pool(name="msk", bufs=1))
        p_ones = ep(tc.tile_pool(name="ones", bufs=1))
        p_io0 = ep(tc.tile_pool(name="io0", bufs=2))
        p_qgt = ep(tc.tile_pool(name="qgt", bufs=2 * NJ))
        p_kgt = ep(tc.tile_pool(name="kgt", bufs=2 * NJ))
        p_vg = ep(tc.tile_pool(name="vg", bufs=KT))
        p_cat = ep(tc.tile_pool(name="cat", bufs=2 * NJ))
        # ---- streaming pools --------------------------------------------
        p_io = ep(tc.tile_pool(name="io", bufs=6))          # qT/kT/vT chunks
        p_pexp = ep(tc.tile_pool(name="pexp", bufs=8))
        p_rec = ep(tc.tile_pool(name="rec", bufs=3))
        p_bc = ep(tc.tile_pool(name="bc", bufs=3))
        p_out = ep(tc.tile_pool(name="out", bufs=4))
        p_ob = ep(tc.tile_pool(name="ob", bufs=2))
        p_ps = ep(tc.tile_pool(name="ps", bufs=1, space="PSUM"))
        p_pss = ep(tc.tile_pool(name="pss", bufs=2, space="PSUM"))
        p_pso2 = ep(tc.tile_pool(name="pso2", bufs=1, space="PSUM"))
        p_pso = ep(tc.tile_pool(name="pso", bufs=2, space="PSUM"))

        # ---- stage constants/weights (outside the timing loop) ----------
        # order matters: DMA queue is served roughly in priority order, so
        # stage what the first matmuls/exps need first.
        w_sb = {}
        b_sb = {}
        for name, wdram, bdram in (("q", wq, bqr), ("k", wk, bkr)):
            t = p_w.tile([128, ND * DH], BF16, tag="w", name=f"w_{name}")
            nc.sync.dma_start(
                t[:].rearrange("p (n c) -> p n c", n=ND),
                wdram.rearrange("(n p) c -> p n c", p=128))
            w_sb[name] = t
            t = p_bias.tile([1, DH], BF16, tag="bias", name=f"b_{name}")
            nc.sync.dma_start(t[:], bdram[:, :])
            b_sb[name] = t
        # prefetch chunk-0 q/k activations (dedicated pool: never recycled)
        io0 = {}
        for name, src_d in (("q", qT), ("k", kT)):
            t = p_io0.tile([128, ND * SC], BF16, tag="io0",
                           name=f"io0_{name}")
            nc.sync.dma_start(
                t[:].rearrange("p (n c) -> p n c", n=ND),
                src_d.rearrange("(n p) s -> p n s", p=128)[:, :, ts(0, SC)])
            io0[name] = t
        msk_t = p_msk.tile([128, 4 * SC], BF16, name="msk_t")
        nc.sync.dma_start(
            msk_t[:].rearrange("p (n c) -> p n c", n=4),
            msk.rearrange("n p c -> p n c"))
        msk_sb = [msk_t[:, ts(r, SC)] for r in range(4)]
        t = p_w.tile([128, ND * DH], BF16, tag="w", name="w_v")
        nc.sync.dma_start(
            t[:].rearrange("p (n c) -> p n c", n=ND),
            wv.rearrange("(n p) c -> p n c", p=128))
        w_sb["v"] = t
        t = p_bias.tile([1, DH], BF16, tag="bias", name="b_v")
        nc.sync.dma_start(t[:], bvr[:, :])
        b_sb["v"] = t
        wo_t = p_wo.tile([128, 2 * D], BF16, name="wo_t")
        nc.sync.dma_start(
            wo_t[:].rearrange("p (n c) -> p n c", n=2),
            wo.rearrange("(n p) c -> p n c", p=128))
        wo_sb = [wo_t[:, ts(kd, D)] for kd in range(2)]
        ones_sb = p_ones.tile([1, SC], BF16)
        nc.vector.memset(ones_sb[:], 1.0)
        # persistent V tiles: ones columns written once, reused every rep
        vg = [p_vg.tile([128, DHA], BF16, tag="vg", name=f"vg{i}")
              for i in range(KT)]
        for i in range(KT):
            vgv = vg[i][:].rearrange("p (g c) -> p g c", c=DK + 1)
            nc.vector.memset(vgv[:, :, DK:DK + 1], 1.0)

        def body():
            # per-chunk persistent products: [128, SC] tiles
            qgt = [[p_qgt.tile([128, SC], BF16, tag="qgt", name=f"qgt{mt}_{j}")
                    for j in range(NJ)] for mt in range(2)]
            kgt = [[p_kgt.tile([128, SC], BF16, tag="kgt", name=f"kgt{mt}_{j}")
                    for j in range(NJ)] for mt in range(2)]
            cat = [[p_cat.tile([128, SC], BF16, tag="cat", name=f"cat{mt}_{j}")
                    for j in range(NJ)] for mt in range(2)]

            def proj_chunk(j):
                # ---- Q/K projection chunk j: out^T layout [DH, SC] ------
                _mark(nc, f"qk{j}")
                ios = {}
                for name, src_d in (("q", qT), ("k", kT)):
                    if j == 0:
                        iot = io0[name]
                    else:
                        iot = p_io.tile([128, ND * SC], BF16, tag="io",
                                        name="io_t")
                        nc.sync.dma_start(
                            iot[:].rearrange("p (n c) -> p n c", n=ND),
                            src_d.rearrange("(n p) s -> p n s",
                                            p=128)[:, :, ts(j, SC)])
                    ios[name] = iot
                for mt in range(2):
                    for name, outt in (("q", qgt), ("k", kgt)):
                        io = [ios[name][:, ts(kd, SC)] for kd in range(ND)]
                        ps = p_ps.tile([128, SC], F32, tag="ps", name="ps_t")
                        for kd in range(ND):
                            nc.tensor.matmul(
                                ps[:], w_sb[name][:, kd * DH + mt * 128:kd * DH + mt * 128 + 128],
                                io[kd], start=(kd == 0),
                                stop=(not with_bias and kd == ND - 1))
                        if with_bias:
                            nc.tensor.matmul(
                                ps[:], b_sb[name][:, ts(mt, 128)],
                                ones_sb[:], start=False, stop=True)
                        nc.vector.tensor_copy(outt[mt][j][:], ps[:])

                # ---- V projection S-tiles of chunk j --------------------
                _mark(nc, f"v{j}")
                iot = p_io.tile([128, ND * SC], BF16, tag="io", name="io_t")
                nc.sync.dma_start(
                    iot[:].rearrange("p (n c) -> p n c", n=ND),
                    vT.rearrange("(n p) s -> p n s", p=128)[:, :, ts(j, SC)])
                io = [iot[:, ts(kd, SC)] for kd in range(ND)]
                for m in range(j * 4, j * 4 + 4):
                    ps = p_ps.tile([128, DH], F32, tag="ps", name="ps_t")
                    for kd in range(ND):
                        nc.tensor.matmul(
                            ps[:], io[kd][:, ts(m - j * 4, 128)],
                            w_sb["v"][:, ts(kd, DH)], start=(kd == 0),
                            stop=(not with_bias and kd == ND - 1))
                    if with_bias:
                        nc.tensor.matmul(
                            ps[:], ones_sb[:, 0:128], b_sb["v"][:],
                            start=False, stop=True)
                    vgv = vg[m][:].rearrange("p (g c) -> p g c", c=DK + 1)
                    psv = ps[:].rearrange("p (g c) -> p g c", c=DK)
                    nc.vector.tensor_copy(vgv[:, :, 0:DK], psv[:])

            def attn_chunk(j):
                # ---- attention for q-chunk j ----------------------------
                _mark(nc, f"at{j}")
                i_last = min(4 * j + 3, KT - 1) if causal else KT - 1
                for hp in range(2):   # head pair = qgt/kgt tile index
                    pso = [p_pso.tile([128, SC], F32, tag="pso",
                                      name="pso_t") for _ in range(2)]
                    for i in range(i_last + 1):
                        # trapezoid: k-tile i only reaches q >= 128*(i-4j)
                        qo = 128 * (i - 4 * j) if (causal and i > 4 * j) \
                            else 0
                        # both heads of the pair share one 2-bank psum
                        # tile; one double-width exp + one masked band mul
                        ps = p_pss.tile([128, 2 * SC], F32, tag="pss",
                                        name="pss_t")
                        for par in range(2):   # even/odd head in pair
                            r0 = 64 * par
                            nc.tensor.matmul(
                                ps[:, par * SC + qo:(par + 1) * SC],
                                kgt[hp][i // 4][r0:r0 + DK,
                                                ts(i % 4, 128)],
                                qgt[hp][j][r0:r0 + DK, qo:SC],
                                start=True, stop=True)
                        pe = p_pexp.tile([128, 2 * SC], BF16, tag="pexp",
                                         name="pe_t")
                        pev = pe[:].rearrange("p (n c) -> p n c", n=2)
                        psv = ps[:].rearrange("p (n c) -> p n c", n=2)
                        nc.scalar.activation(
                            pev[:, :, qo:SC], psv[:, :, qo:SC],
                            mybir.ActivationFunctionType.Exp,
                            scale=SCALE)
                        if causal and i >= 4 * j:
                            mkv = msk_sb[i - 4 * j][:, qo:qo + 128]
                            nc.vector.tensor_mul(
                                pev[:, :, qo:qo + 128],
                                pev[:, :, qo:qo + 128],
                                mkv.rearrange("p (n c) -> p n c", n=1)
                                .broadcast_to((128, 2, 128)))
                        for par in range(2):
                            h = 2 * hp + par
                            nc.tensor.matmul(
                                pso[par][0:65, qo:SC],
                                vg[i][:, h * 65:h * 65 + 65],
                                pe[:, par * SC + qo:(par + 1) * SC],
                                start=(i == 0),
                                stop=(i == i_last), skip_group_check=True)
                    for par in range(2):
                        r0 = 64 * par
                        rec = p_rec.tile([1, SC], F32, tag="rec",
                                         name="rec_t")
                        nc.vector.reciprocal(rec[:], pso[par][64:65, :])
                        bc = p_bc.tile([DK, SC], F32, tag="bc", name="bc_t")
                        nc.gpsimd.partition_broadcast(bc[:], rec[:])
                        nc.vector.tensor_mul(
                            cat[hp][j][r0:r0 + DK, :],
                            pso[par][0:DK, :], bc[:])

                # ---- output projection chunk j: O^T partial [D, SC] -----
                _mark(nc, f"o{j}")
                last_chunk = (j == NJ - 1)
                if not last_chunk:
                    ob = p_ob.tile([128, ND * SC], BF16, tag="ob",
                                   name="ob_t")
                for mtd in range(ND):
                    # last chunk is the kernel tail: alternate psum banks
                    # (proj pool is idle by now) and split copies DVE/ACT
                    # so the 8 groups pipeline instead of serializing
                    if last_chunk and mtd % 2:
                        ps = p_ps.tile([128, SC], F32, tag="ps", name="ps_t")
                    else:
                        ps = p_pso2.tile([128, SC], F32, tag="pso2",
                                         name="pso2_t")
                    for kd in range(2):
                        nc.tensor.matmul(
                            ps[:], wo_sb[kd][:, ts(mtd, 128)],
                            cat[kd][j][:], start=(kd == 0), stop=(kd == 1))
                    if last_chunk:
                        # tail: small copies + per-tile DMAs pipeline out
                        ot = p_out.tile([128, SC], BF16, tag="out",
                                        name="ot_t")
                        if mtd % 2:
                            nc.scalar.activation(
                                ot[:], ps[:],
                                mybir.ActivationFunctionType.Identity)
                        else:
                            nc.vector.tensor_copy(ot[:], ps[:])
                        nc.sync.dma_start(oT[ts(mtd, 128), ts(j, SC)], ot[:])
                    else:
                        nc.vector.tensor_copy(ob[:, ts(mtd, SC)], ps[:])
                if not last_chunk:
                    # one batched DMA for the whole chunk column
                    nc.sync.dma_start(
                        oT.rearrange("(n p) s -> p n s",
                                     p=128)[:, :, ts(j, SC)],
                        ob[:].rearrange("p (n c) -> p n c", n=ND))

            if causal:
                # pipelined: attention j only needs K/V chunks <= j
                for j in range(NJ):
                    proj_chunk(j)
                    attn_chunk(j)
            else:
                # attention needs ALL K/V chunks: project everything first
                for j in range(NJ):
                    proj_chunk(j)
                for j in range(NJ):
                    attn_chunk(j)

        if reps > 1:
            import concourse.mybir as _mb
            with tc.For_i(0, reps, 1,
                          hint_engines=(_mb.EngineType.PE,
                                        _mb.EngineType.Activation,
                                        _mb.EngineType.DVE,
                                        _mb.EngineType.SP,
                                        _mb.EngineType.Pool)):
                body()
        else:
            body()


def _pack_core_inputs(inputs_np, c, masks):
    b, g = c // 4, c % 4
    q, k, v = inputs_np["q"], inputs_np["k"], inputs_np["v"]
    cs = slice(DH * g, DH * (g + 1))
    return {
        "qT": np.ascontiguousarray(q[b].T).astype(bf16),
        "kT": np.ascontiguousarray(k[b].T).astype(bf16),
        "vT": np.ascontiguousarray(v[b].T).astype(bf16),
        "wq": np.ascontiguousarray(inputs_np["Wq"][:, cs]).astype(bf16),
        "wk": np.ascontiguousarray(inputs_np["Wk"][:, cs]).astype(bf16),
        "wv": np.ascontiguousarray(inputs_np["Wv"][:, cs]).astype(bf16),
        "wo": np.ascontiguousarray(inputs_np["Wo"][cs, :]).astype(bf16),
        "bqr": inputs_np["bq"][cs].reshape(1, DH).astype(bf16),
        "bkr": inputs_np["bk"][cs].reshape(1, DH).astype(bf16),
        "bvr": inputs_np["bv"][cs].reshape(1, DH).astype(bf16),
        "msk": masks,
    }


def _build_masks():
    p = np.arange(128)[:, None]
    f = np.arange(SC)[None, :]
    m = np.stack([(f >= 128 * r + p) for r in range(4)]).astype(bf16)
    return np.ascontiguousarray(m)


_PROGRAM_CACHE = {}


def get_program(causal: bool, reps: int = 1, with_bias: bool = True):
    key = (causal, reps, with_bias)
    if key not in _PROGRAM_CACHE:
        _PROGRAM_CACHE[key] = build_program(causal, reps, with_bias)
    return _PROGRAM_CACHE[key]


def kernel(q, k, v, Wq, bq, Wk, bk, Wv, bv, Wo, bo, use_causal_mask):
    inputs_np = {
        "q": np.asarray(q, np.float32), "k": np.asarray(k, np.float32),
        "v": np.asarray(v, np.float32),
        "Wq": np.asarray(Wq, np.float32), "bq": np.asarray(bq, np.float32),
        "Wk": np.asarray(Wk, np.float32), "bk": np.asarray(bk, np.float32),
        "Wv": np.asarray(Wv, np.float32), "bv": np.asarray(bv, np.float32),
        "Wo": np.asarray(Wo, np.float32), "bo": np.asarray(bo, np.float32),
    }
    causal = bool(int(np.asarray(use_causal_mask).item()))
    wb = any(np.any(inputs_np[k]) for k in ("bq", "bk", "bv"))
    nc = get_program(causal, 1, wb)
    masks = _build_masks()
    in_maps = [_pack_core_inputs(inputs_np, c, masks) for c in range(N_CORES)]
    res = run_bass_kernel_spmd(nc, in_maps, list(range(N_CORES))).results
    out = np.zeros((B, S, D), np.float32)
    for c in range(N_CORES):
        out[c // 4] += res[c]["oT"].T.astype(np.float32)
    out += inputs_np["bo"][None, None, :]
    return out
